# revision 3
# baseline (speedup 1.0000x reference)
"""Trainium2 Bass kernel for nn_MeaMDensity22 (gnn_message_passing), v2.

Data-parallel over molecules: 2 molecules per NeuronCore, 8 cores.

Per-core device program (KP = max neighbor count, padded to 32):
  * Host sorts each molecule's pairs by center atom into a [KP, A] grid and
    ships dvec (bf16), d2 (fp32), d2^T (bf16), and a block-diagonal
    wf-selector (bf16).  Padding slots get d2 = 1e8 so exp() kills them --
    no mask tensor at all.
  * Cutoff cosine 0.5*(1+cos(pi*min(d/C,1))) == poly3(min(d2/C^2,1)):
    cos(pi*sqrt(u)) is analytic in u, a cubic fits to 1.3e-3.  No Sin
    activation -> only two ACT table loads (sqrt set, exp set), both hidden.
  * rsq = Sqrt(reciprocal_approx_fast(d2)) -- one ACT op.
  * exp argument (wf_g * d2) built ON THE PE: stationary = d2^T slice,
    moving = block-diag selector; lands in PSUM in 32-atom chunks; ACT Exp
    reads PSUM and writes bf16 gauss to SBUF.
  * Angular rows (3 + 9) on DVE in bf16 (2x mode).
  * Segment-sum = per-atom matmul gauss^T @ ang in bf16 into [32, 384]
    PSUM bank tiles; Square (ACT/DVE/Pool) then per-bank strided reduces
    (DVE) produce dens in [32g, m, o, a] layout; host transposes.
"""

import math
import os
import sys

import numpy as np

sys.path.insert(0, "/opt/trn_rl_repo")

A = 128          # atoms per molecule
G = 32           # gaussians
E = 3            # species
LDIM = 12        # angular rows (3 + 9)
CUTOFF = 5.0
NCORES = 8
NMOL = 2         # molecules per core
PAD_D2 = 1.0e8   # padded slots: gauss = exp(wf*PAD_D2) = 0

# Fit 0.5*(1+cos(pi*sqrt(u))) = (1-u)*r(u) on [0,1], r cubic (max err 5e-5).
# The (1-u) factor makes cut(u>=1) EXACTLY zero -- pairs beyond the cutoff
# must not leak through the wide gaussians.
_u = np.linspace(0.0, 1.0, 20001)
_y = 0.5 * (1.0 + np.cos(np.pi * np.sqrt(_u)))
_A = np.stack([(1.0 - _u) * _u ** k for k in range(4)], 1)
_R0, _R1, _R2, _R3 = [float(c) for c in np.linalg.lstsq(_A, _y, rcond=None)[0]]


def _bf16(x):
    import ml_dtypes
    return np.asarray(x, np.float32).astype(ml_dtypes.bfloat16)


def _prep_molecule(coords_b, shifts_b, idx_b, KP):
    """Sorted center-grid arrays for one molecule.

    Returns dvec_g [KP,A,3] f32, d2_g [KP,A] f32 (padding = PAD_D2).
    """
    i = np.asarray(idx_b[0], np.int64)
    j = np.asarray(idx_b[1], np.int64)
    order = np.argsort(i, kind="stable")
    i_s = i[order]
    counts = np.bincount(i, minlength=A)
    starts = np.zeros(A, np.int64)
    starts[1:] = np.cumsum(counts)[:-1]
    rows = np.arange(i.shape[0], dtype=np.int64) - starts[i_s]
    cols = i_s

    dvec = coords_b[i] - coords_b[j] + shifts_b          # (P, 3) f32
    valid = np.all(shifts_b > -1e9, axis=1)
    d2 = (dvec * dvec).sum(1)
    d2 = np.where(valid, d2, PAD_D2)

    dvec_g = np.zeros((KP, A, 3), np.float32)
    d2_g = np.full((KP, A), PAD_D2, np.float32)
    dvec_g[rows, cols] = dvec[order]
    d2_g[rows, cols] = d2[order]
    return dvec_g, d2_g


def _build_program(KP, uniform_w):
    import concourse.bass as bass
    import concourse.bacc as bacc
    import concourse.tile as tile
    from concourse import mybir

    f32 = mybir.dt.float32
    bf16 = mybir.dt.bfloat16
    AF = mybir.ActivationFunctionType
    OP = mybir.AluOpType
    X = mybir.AxisListType.X

    NB = 4                      # psum bank-groups of 32 atoms per molecule
    AB = 32                     # atoms per bank group
    NSEL = 1 if uniform_w else NMOL * NB

    nc = bacc.Bacc("TRN2")

    dvec_d = nc.dram_tensor("dvec", [KP, NMOL * A * 3], bf16, kind="ExternalInput")
    d2_d = nc.dram_tensor("d2", [KP, NMOL * A], f32, kind="ExternalInput")
    d2t_d = nc.dram_tensor("d2t", [AB, NMOL * NB * KP], bf16, kind="ExternalInput")
    sel_d = nc.dram_tensor("sel", [AB, NSEL * AB * G], bf16, kind="ExternalInput")
    out_d = nc.dram_tensor("dens", [G, NMOL * 2 * A], f32, kind="ExternalOutput")

    with tile.TileContext(nc) as tc:
        import contextlib
        ctx = contextlib.ExitStack()
        with ctx:
            pool = ctx.enter_context(tc.tile_pool(name="p", bufs=1))
            ps_targ = ctx.enter_context(
                tc.tile_pool(name="ps_targ", bufs=3, space="PSUM")
            )
            ps_sw = ctx.enter_context(
                tc.tile_pool(name="ps_sw", bufs=2, space="PSUM")
            )

            # ---- input DMAs (issue order = need order: d2 -> d2t -> sel
            # -> dvec; the DGE queue serializes at ~650ns per transfer) ----
            d2_t = pool.tile([KP, NMOL, A], f32, name="d2_t")
            nc.sync.dma_start(
                out=d2_t, in_=d2_d[:].rearrange("k (m a) -> k m a", m=NMOL)
            )
            d2t_t = pool.tile([AB, NMOL, NB, KP], bf16, name="d2t_t")
            nc.sync.dma_start(
                out=d2t_t,
                in_=d2t_d[:].rearrange("a (m b k) -> a m b k", m=NMOL, b=NB),
            )
            sel_t = pool.tile([AB, NSEL, AB * G], bf16, name="sel_t")
            nc.sync.dma_start(
                out=sel_t,
                in_=sel_d[:].rearrange("a (s x) -> a s x", s=NSEL),
            )
            dvec_t = pool.tile([KP, NMOL, A, 3], bf16, name="dvec_t")
            nc.sync.dma_start(
                out=dvec_t,
                in_=dvec_d[:].rearrange("k (m a c) -> k m a c", m=NMOL, c=3),
            )

            # ---- DVE scalar chain (f32): ri2, then cut poly via Pool ----
            ri2 = pool.tile([KP, NMOL, A], f32, name="ri2")
            nc.vector.reciprocal_approx_fast(ri2[:], d2_t[:])
            rsq = pool.tile([KP, NMOL, A], bf16, name="rsq")
            nc.scalar.activation(rsq[:], ri2[:], AF.Sqrt)   # sqrt table set

            # u = min(d2/C^2, 1)  (bf16 out, 2x TS)
            u_t = pool.tile([KP, NMOL, A], bf16, name="u_t")
            nc.vector.tensor_scalar(
                out=u_t[:], in0=d2_t[:], scalar1=1.0 / (CUTOFF * CUTOFF),
                scalar2=1.0, op0=OP.mult, op1=OP.min,
            )
            # cutoff = (1-u) * r(u), r cubic by Horner -- DVE bf16 (TS 4x,
            # TT 2x; a serialized Pool chain here sat on the critical path)
            w_t = pool.tile([KP, NMOL, A], bf16, name="w_t")
            nc.vector.tensor_scalar(
                out=w_t[:], in0=u_t[:], scalar1=-1.0, scalar2=1.0,
                op0=OP.mult, op1=OP.add,
            )
            h1 = pool.tile([KP, NMOL, A], bf16, name="h1")
            nc.vector.tensor_scalar(
                out=h1[:], in0=u_t[:], scalar1=_R3, scalar2=_R2,
                op0=OP.mult, op1=OP.add,
            )
            m1 = pool.tile([KP, NMOL, A], bf16, name="m1")
            nc.vector.tensor_tensor(out=m1[:], in0=h1[:], in1=u_t[:], op=OP.mult)
            a1 = pool.tile([KP, NMOL, A], bf16, name="a1")
            nc.vector.tensor_scalar(
                out=a1[:], in0=m1[:], scalar1=_R1, scalar2=None, op0=OP.add
            )
            m2 = pool.tile([KP, NMOL, A], bf16, name="m2")
            nc.vector.tensor_tensor(out=m2[:], in0=a1[:], in1=u_t[:], op=OP.mult)
            a2 = pool.tile([KP, NMOL, A], bf16, name="a2")
            nc.vector.tensor_scalar(
                out=a2[:], in0=m2[:], scalar1=_R0, scalar2=None, op0=OP.add
            )
            cut = pool.tile([KP, NMOL, A], bf16, name="cut")
            nc.vector.tensor_tensor(out=cut[:], in0=a2[:], in1=w_t[:], op=OP.mult)

            # ---- angular rows (DVE, bf16 2x) ----
            unit = pool.tile([KP, NMOL, A, 3], bf16, name="unit")
            nc.vector.tensor_tensor(
                out=unit[:], in0=dvec_t[:],
                in1=rsq[:].unsqueeze(3).broadcast_to([KP, NMOL, A, 3]),
                op=OP.mult,
            )
            ang = pool.tile([KP, NMOL, A, LDIM], bf16, name="ang")
            nc.vector.tensor_tensor(
                out=ang[:, :, :, 0:3], in0=unit[:],
                in1=cut[:].unsqueeze(3).broadcast_to([KP, NMOL, A, 3]),
                op=OP.mult,
            )
            # ang9[i,j] = unit_i * ang3_j; broadcast operands forfeit DVE 2x,
            # so split j: DVE takes j=0,1 and Pool takes j=2 in parallel.
            ang9v = ang[:, :, :, 3:12].rearrange("k m a (i j) -> k m a i j", i=3)
            nc.vector.tensor_tensor(
                out=ang9v[:, :, :, :, 0:2],
                in0=unit[:].unsqueeze(4).broadcast_to([KP, NMOL, A, 3, 2]),
                in1=ang[:, :, :, 0:2].unsqueeze(3).broadcast_to([KP, NMOL, A, 3, 2]),
                op=OP.mult,
            )
            nc.gpsimd.tensor_tensor(
                out=ang9v[:, :, :, :, 2:3],
                in0=unit[:].unsqueeze(4).broadcast_to([KP, NMOL, A, 3, 1]),
                in1=ang[:, :, :, 2:3].unsqueeze(3).broadcast_to([KP, NMOL, A, 3, 1]),
                op=OP.mult,
            )

            # ---- per 32-atom chunk: targ matmul -> exp -> sumw matmuls ----
            gauss = pool.tile([KP, NMOL, A, G], bf16, name="gauss")
            sq_sb = pool.tile([G, NMOL, NB, AB * LDIM], bf16, name="sq_sb")
            dens_pre = pool.tile([G, NMOL, 2, A], f32, name="dens_pre")
            m1_sw = []

            for m in range(NMOL):
                for b in range(NB):
                    s = 0 if uniform_w else m * NB + b
                    targ_ps = ps_targ.tile(
                        [KP, AB * G], f32, tag="targ", name=f"targ_{m}_{b}"
                    )
                    # matmul out must fit one PSUM bank (512 f32): two halves
                    for h in range(2):
                        nc.tensor.matmul(
                            targ_ps[:, h * 512:(h + 1) * 512],
                            d2t_t[:, m, b, :],              # [32, KP] stationary
                            sel_t[:, s, h * 512:(h + 1) * 512],  # [32, 512]
                            start=True, stop=True,
                        )
                    # exp chunk: PSUM -> SBUF bf16 (exp table set)
                    nc.scalar.activation(
                        gauss[:, m, b * AB:(b + 1) * AB, :],
                        targ_ps[:].rearrange("k (a g) -> k a g", g=G),
                        AF.Exp,
                    )
                    # sumw: per-atom matmuls into one bank tile [32, 384]
                    sw_ps = ps_sw.tile([G, AB * LDIM], f32, tag="sw",
                                       name=f"sw_{m}_{b}")
                    for ai in range(AB):
                        a = b * AB + ai
                        nc.tensor.matmul(
                            sw_ps[:, ai * LDIM:(ai + 1) * LDIM],
                            gauss[:, m, a, :],
                            ang[:, m, a, :],
                            start=True, stop=True,
                        )
                    # square: TensorTensor may read only ONE psum input, so
                    # m0 copies psum->sbuf bf16 on DVE and squares there
                    # (all under the exp window); m1's squares go on ACT but
                    # are DEFERRED after the last exp chunk so they don't
                    # interleave into the in-order exp chain.
                    if m == 0:
                        dst = sq_sb[:, m, b, :]
                        cp = pool.tile([G, AB * LDIM], bf16, tag="cp",
                                       name=f"cp_{m}_{b}", bufs=2)
                        nc.vector.tensor_copy(out=cp[:], in_=sw_ps[:])
                        nc.vector.tensor_tensor(
                            out=dst, in0=cp[:], in1=cp[:], op=OP.mult
                        )
                    else:
                        m1_sw.append((b, sw_ps))
                if m == 0:
                    # batched reduces for m0 (not latency-critical)
                    v = sq_sb[:, 0, :, :].rearrange(
                        "g b (a l) -> g (b a) l", l=LDIM
                    )
                    nc.vector.tensor_reduce(
                        out=dens_pre[:, 0, 0, :].unsqueeze(2),
                        in_=v[:, :, 0:3], axis=X, op=OP.add,
                    )
                    nc.vector.tensor_reduce(
                        out=dens_pre[:, 0, 1, :].unsqueeze(2),
                        in_=v[:, :, 3:12], axis=X, op=OP.add,
                    )

            # m1 critical tail: banks 0,1 square via DVE copy (data is ready
            # well before the exp chain ends); banks 2,3 square on ACT right
            # after the last exp. Per-bank reduces pipeline behind each.
            for b, sw_ps in m1_sw:
                dst = sq_sb[:, 1, b, :]
                nc.scalar.activation(dst, sw_ps[:], AF.Square)
                v = dst.rearrange("g (a l) -> g a l", l=LDIM)
                nc.vector.tensor_reduce(
                    out=dens_pre[:, 1, 0, b * AB:(b + 1) * AB].unsqueeze(2),
                    in_=v[:, :, 0:3], axis=X, op=OP.add,
                )
                nc.vector.tensor_reduce(
                    out=dens_pre[:, 1, 1, b * AB:(b + 1) * AB].unsqueeze(2),
                    in_=v[:, :, 3:12], axis=X, op=OP.add,
                )

            nc.sync.dma_start(
                out=out_d[:],
                in_=dens_pre[:].rearrange("g m o a -> g (m o a)"),
            )

    nc.compile()
    return nc


_PROGRAM_CACHE = {}


def _get_program(KP, uniform_w):
    key = (KP, uniform_w)
    if key not in _PROGRAM_CACHE:
        _PROGRAM_CACHE[key] = _build_program(KP, uniform_w)
    return _PROGRAM_CACHE[key]


def kernel(coordinates, shifts, ang_offsets, atom_index, species, numatoms):
    from concourse.bass_utils import run_bass_kernel_spmd

    coordinates = np.asarray(coordinates, np.float32)
    shifts = np.asarray(shifts, np.float32)
    ang_offsets = np.asarray(ang_offsets, np.float32)
    atom_index = np.asarray(atom_index)
    species = np.asarray(species)

    B, A_, _ = coordinates.shape
    assert A_ == A and B == NCORES * NMOL

    KP = 32
    for b in range(B):
        cnts = np.bincount(np.asarray(atom_index[b, 0], np.int64), minlength=A)
        KP = max(KP, int(cnts.max()))
    KP = min(128, int(math.ceil(KP / 32.0) * 32))
    uniform_w = bool(np.all(ang_offsets == ang_offsets[0:1]))

    nc = _get_program(KP, uniform_w)

    wf = -0.5 / (ang_offsets * ang_offsets)          # (E, G)

    # selector sel[loc, s, loc*G:(loc+1)*G] = wf[species(atom)], block-diag
    # [32, 32*G]; uniform species -> one pattern serves every 32-atom chunk.
    sp_mol = species.reshape(B, A)
    NB, AB = 4, 32
    NSEL = 1 if uniform_w else NMOL * NB

    in_maps = []
    for c in range(NCORES):
        dvec_all = np.zeros((KP, NMOL, A, 3), np.float32)
        d2_all = np.full((KP, NMOL, A), PAD_D2, np.float32)
        for m in range(NMOL):
            b = c * NMOL + m
            dvec_g, d2_g = _prep_molecule(
                coordinates[b], shifts[b], atom_index[b], KP
            )
            dvec_all[:, m] = dvec_g
            d2_all[:, m] = d2_g
        # [AB, NMOL, NB, KP]: d2t[loc, m, b, k] = d2[k, m, b*AB+loc]
        d2t_all = np.transpose(
            d2_all.reshape(KP, NMOL, NB, AB), (3, 1, 2, 0)
        ).copy()

        sel_all = np.zeros((AB, NSEL, AB * G), np.float32)
        for s in range(NSEL):
            m, bk = divmod(s, NB) if not uniform_w else (0, 0)
            b = c * NMOL + m
            for loc in range(AB):
                atom = bk * AB + loc
                w = wf[sp_mol[b, atom]] if not uniform_w else wf[0]
                sel_all[loc, s, loc * G:(loc + 1) * G] = w

        in_maps.append(
            {
                "dvec": _bf16(dvec_all.reshape(KP, NMOL * A * 3)),
                "d2": d2_all.reshape(KP, NMOL * A),
                "d2t": _bf16(d2t_all.reshape(AB, NMOL * NB * KP)),
                "sel": _bf16(sel_all.reshape(AB, NSEL * AB * G)),
            }
        )

    trace = bool(int(os.environ.get("KERNEL_TRACE", "0")))
    res = run_bass_kernel_spmd(
        nc, in_maps, core_ids=list(range(NCORES)), trace=trace
    )
    if trace and res.exec_time_ns is not None:
        print(f"HW exec time: {res.exec_time_ns} ns")

    out = np.zeros((B * A, 2 * G), np.float32)
    for c in range(NCORES):
        dens = np.asarray(res.results[c]["dens"], np.float32)  # [G, NMOL*2*A]
        d = dens.reshape(G, NMOL, 2, A)
        for m in range(NMOL):
            b = c * NMOL + m
            # out[b*A + a, o*G + g] = d[g, m, o, a]
            out[b * A:(b + 1) * A, :] = (
                d[:, m].transpose(2, 1, 0).reshape(A, 2 * G)
            )
    return out
